# revision 15
# baseline (speedup 1.0000x reference)
"""Multi-head attention (B=4, S=2048, D=768, H=12, d=64) on 8 trn2 NeuronCores.

Sharding: core c handles batch b = c//2 and head-group g = c%2 (6 heads each).
Per core: column-parallel QKV projections, full attention for its 6 heads,
row-parallel output projection; the two partial outputs per batch are reduced
on the host (+ bo and the bv @ wo correction, exact because softmax rows sum
to 1).

Dataflow (v3):
- x and W arrive bf16 (host-cast); projections produce bf16 qt/kt
  [128(=2 heads' d), S] and bf16 vt[st] [128 s, 6 x (64 V | ones)].
- 12 chunks: (q-half sc in {0,1}) x (head h in 0..5). Per chunk, sk walks
  the 16 key tiles: scores [128 k, 1024 q] fp32 PSUM (2 x N=512 matmuls),
  ONE exp on ACT (scale=1/8) -> bf16 e tile.
- ctx V-stationary: lhsT = [V_h | ones] [128k, 65] (65-col weight load
  hides under the 512-wide streams), rhs = e halves [128k, 512] -> PSUM
  [65, 1024] accumulated over sk; row 64 accumulates the softmax
  denominator for free.
- Drain per chunk: DVE reciprocal of row 64 -> bf16 r [1, 1024]; PE
  broadcast matmul (ones [1,64] lhsT) -> [64, 512] x2; DVE multiply
  normalizes into ctx_m[m] rows 0:64 (even heads) or a staging tile that a
  partition-shift DMA moves to rows 64:128 (odd heads).
- Out-projection: bf16 ctx_m lhsT x bf16 wo in [128,512]+[128,256] rounds
  through the aux PSUM bank, interleaved into later chunks (as are the
  deferred QKV projection rounds, scheduled just-in-time).

PSUM (8 banks): scores 2x[128,1024] = 4, ctx [65,1024] = 2, bcast = 1,
aux = 1.
"""
import sys

for _p in ("/opt/trn_rl_repo", "/root/.axon_site/_ro/trn_rl_repo"):
    if _p not in sys.path:
        sys.path.append(_p)

import numpy as np

import concourse.bass as bass  # noqa: F401
import concourse.bacc as bacc
import concourse.mybir as mybir
import concourse.tile as tile
from concourse.bass_utils import run_bass_kernel_spmd

B, S, D = 4, 2048, 768
NUM_HEADS, HEAD = 12, 64
NCORES = 8
HPC = NUM_HEADS // 2          # 6 heads per core
MC = HPC * HEAD               # 384 per-core projection cols
KT = D // 128                 # 6 contraction k-tiles
MT = MC // 128                # 3 head-pair tiles
ST = S // 128                 # 16 key tiles
CW = 1024                     # q-chunk width
NCH = S // CW                 # 2 q-chunks
XC = 512                      # x column chunk for projection rounds

F32 = mybir.dt.float32
F32R = mybir.dt.float32r
BF16 = mybir.dt.bfloat16
EXP = mybir.ActivationFunctionType.Exp
MULT = mybir.AluOpType.mult

_NC = None
LAST_RESULTS = None
_LAST_IN_MAPS = None
_DONE = object()


def _build(loop=None):
    nc = bacc.Bacc("TRN2", target_bir_lowering=False, debug=False,
                   num_devices=NCORES)
    xqt = nc.declare_dram_parameter("xqt", [D, S], BF16, isOutput=False)
    xkt = nc.declare_dram_parameter("xkt", [D, S], BF16, isOutput=False)
    xvt = nc.declare_dram_parameter("xvt", [D, S], BF16, isOutput=False)
    wq = nc.declare_dram_parameter("wq", [D, MC], BF16, isOutput=False)
    wk = nc.declare_dram_parameter("wk", [D, MC], BF16, isOutput=False)
    wv = nc.declare_dram_parameter("wv", [D, MC], BF16, isOutput=False)
    wo = nc.declare_dram_parameter("wo", [MC, D], BF16, isOutput=False)
    bq = nc.declare_dram_parameter("bq", [MC], F32, isOutput=False)
    bk = nc.declare_dram_parameter("bk", [MC], F32, isOutput=False)
    out = nc.declare_dram_parameter("out", [S, D], F32, isOutput=True)

    with tile.TileContext(nc) as tc:
        if loop:
            with tc.For_i(0, loop, 1):
                _emit(nc, tc, xqt, xkt, xvt, wq, wk, wv, wo, bq, bk, out)
        else:
            _emit(nc, tc, xqt, xkt, xvt, wq, wk, wv, wo, bq, bk, out)
    nc.compile()
    return nc


def _emit(nc, tc, xqt, xkt, xvt, wq, wk, wv, wo, bq, bk, out):
    ctx_lp = nc.allow_low_precision(
        reason="bf16 attention pipeline; accumulation stays fp32 in PSUM")
    ctx_lp.__enter__()
    with (
        tc.tile_pool(name="wp", bufs=1) as w_pool,
        tc.tile_pool(name="xp", bufs=1) as x_pool,
        tc.tile_pool(name="qtp", bufs=1) as qt_pool,
        tc.tile_pool(name="ktp", bufs=1) as kt_pool,
        tc.tile_pool(name="vp", bufs=ST) as v_pool,
        tc.tile_pool(name="ep", bufs=1) as e_pool,
        tc.tile_pool(name="cnp", bufs=1) as cn_pool,
        tc.tile_pool(name="cmp", bufs=1) as cm_pool,
        tc.tile_pool(name="op", bufs=1) as o_pool,
        tc.tile_pool(name="psS", bufs=1, space="PSUM") as psS,
        tc.tile_pool(name="psC", bufs=1, space="PSUM") as psC,
        tc.tile_pool(name="psB", bufs=1, space="PSUM") as psB,
        tc.tile_pool(name="psX", bufs=1, space="PSUM") as psX,
    ):
        # ---- persistent SBUF tiles ----
        w_sb = {}
        for name in ("wv", "wk", "wq"):
            w_sb[name] = w_pool.tile([128, KT, MC], BF16, tag=name,
                                     name=f"w_{name}")
        wo_sb = w_pool.tile([128, MT, D], BF16, tag="wo")
        bq_sb = w_pool.tile([128, MT], F32, tag="bq")
        bk_sb = w_pool.tile([128, MT], F32, tag="bk")
        ones_bc = w_pool.tile([1, HEAD], BF16, tag="ones")

        qt = [qt_pool.tile([128, S], BF16, tag=f"qt{m}", name=f"qt{m}")
              for m in range(MT)]
        kt = [kt_pool.tile([128, S], BF16, tag=f"kt{m}", name=f"kt{m}")
              for m in range(MT)]
        vt = [v_pool.tile([128, HPC, HEAD + 1], BF16, tag="v",
                          name=f"vt{st}") for st in range(ST)]
        ctx_m = [cm_pool.tile([128, S], BF16, tag=f"cm{m}", name=f"ctxm{m}")
                 for m in range(MT)]

        # ---- weight / bias DMAs (ACT queue; idle during prologue) ----
        for name, w in (("wv", wv), ("wk", wk), ("wq", wq)):
            nc.scalar.dma_start(out=w_sb[name],
                                in_=w[:].rearrange("(n k) m -> k n m", k=128))
        nc.scalar.dma_start(out=wo_sb,
                            in_=wo[:].rearrange("(t p) o -> p t o", p=128))
        nc.scalar.dma_start(out=bq_sb,
                            in_=bq[:].rearrange("(t p) -> p t", p=128))
        nc.scalar.dma_start(out=bk_sb,
                            in_=bk[:].rearrange("(t p) -> p t", p=128))

        nc.gpsimd.memset(ones_bc, 1.0)
        for st in range(ST):
            nc.gpsimd.memset(vt[st][:, :, HEAD:HEAD + 1], 1.0)

        # ---- x chunk loads (single DMA each) ----
        x_dram = {"v": xvt, "k": xkt, "q": xqt}
        x_bufs = {"v": 2, "k": 4, "q": 4}
        x_sb = {}

        def load_x(inp, c, eng=None):
            t_ = x_pool.tile([128, KT, XC], BF16, tag=f"x{inp}",
                             bufs=x_bufs[inp], name=f"x{inp}{c}")
            x_sb[(inp, c)] = t_
            cols = slice(c * XC, (c + 1) * XC)
            (eng or nc.sync).dma_start(
                out=t_,
                in_=x_dram[inp][:, cols].rearrange("(n k) s -> k n s", k=128))

        # ---- projection rounds ----
        def v_round(st, ps_pool, tag):
            c = (st * 128) // XC
            s_in_c = (st * 128) % XC
            xs = x_sb[("v", c)]
            ps = ps_pool.tile([128, 512], F32, tag=tag, name=f"psv{st}",
                              bufs=2 if tag == "s" else None)
            for k in range(KT):
                nc.tensor.matmul(ps[:, 0:MC],
                                 xs[:, k, s_in_c:s_in_c + 128],
                                 w_sb["wv"][:, k, :],
                                 start=(k == 0), stop=(k == KT - 1))
            psv = ps[:, 0:MC].rearrange("p (h d) -> p h d", d=HEAD)
            nc.vector.tensor_copy(vt[st][:, :, 0:HEAD], psv)

        def qk_round(inp, m, c, ps_pool, tag):
            xs = x_sb[(inp, c)]
            dst = qt[m] if inp == "q" else kt[m]
            bias = bq_sb if inp == "q" else bk_sb
            wn = "wq" if inp == "q" else "wk"
            ps = ps_pool.tile([128, 512], F32, tag=tag, name=f"ps{inp}{m}{c}",
                              bufs=2 if tag == "s" else None)
            for k in range(KT):
                nc.tensor.matmul(ps,
                                 w_sb[wn][:, k, m * 128:(m + 1) * 128],
                                 xs[:, k, :],
                                 start=(k == 0), stop=(k == KT - 1))
            nc.vector.tensor_scalar_add(dst[:, c * XC:(c + 1) * XC], ps,
                                        bias[:, m:m + 1])

        # ---- out-projection rounds (generator; one yield per round) ----
        def outproj_rounds(sc, slots=None):
            slots = slots or [(psX, "aux")]
            si = 0
            for st4 in range(CW // 128):
                s0 = sc * CW + st4 * 128
                o_sb = o_pool.tile([128, D], F32, tag="osb", bufs=2,
                                   name=f"osb{sc}{st4}")
                for n0, nw in ((0, 512), (512, 256)):
                    sp, stg = slots[si % len(slots)]
                    si += 1
                    ps_o = sp.tile([128, 512], F32, tag=stg,
                                   name=f"pso{sc}{st4}{n0}",
                                   bufs=2 if stg == "s" else None)
                    for m in range(MT):
                        nc.tensor.matmul(
                            ps_o[:, 0:nw],
                            ctx_m[m][:, s0:s0 + 128],
                            wo_sb[:, m, n0:n0 + nw],
                            start=(m == 0), stop=(m == MT - 1))
                    nc.vector.tensor_copy(o_sb[:, n0:n0 + nw], ps_o[:, 0:nw])
                    if n0 == 512:
                        nc.sync.dma_start(out=out[s0:s0 + 128, :], in_=o_sb)
                    yield True

        # ---- chunk drain (2 stages so PE never waits on the recips) ----
        def drain_stage0(sc, h, ps_c):
            r = cn_pool.tile([1, CW], BF16, tag="r", bufs=2, name=f"r{sc}{h}")
            for half in range(2):
                qs = slice(half * 512, (half + 1) * 512)
                nc.vector.reciprocal(r[:, qs], ps_c[HEAD:HEAD + 1, qs])
            return r

        def drain_stage1(sc, h, ps_c, r):
            m, h2 = h // 2, h % 2
            cno = None
            if h2 == 1:
                cno = cn_pool.tile([64, CW], BF16, tag="cno", bufs=2,
                                   name=f"cno{sc}{h}")
            for half in range(2):
                qs = slice(half * 512, (half + 1) * 512)
                ps_b = psB.tile([64, 512], F32, tag="b",
                                name=f"psb{sc}{h}{half}")
                nc.tensor.matmul(ps_b, ones_bc, r[:, qs],
                                 start=True, stop=True,
                                 skip_group_check=True)
                bc_sb = cn_pool.tile([64, 512], BF16, tag="bc", bufs=2,
                                     name=f"bc{sc}{h}{half}")
                nc.vector.tensor_copy(bc_sb, ps_b)
                if h2 == 0:
                    dstq = slice(sc * CW + half * 512,
                                 sc * CW + (half + 1) * 512)
                    nc.vector.tensor_tensor(ctx_m[m][0:64, dstq],
                                            ps_c[0:64, qs], bc_sb, op=MULT)
                else:
                    nc.vector.tensor_tensor(cno[:, qs],
                                            ps_c[0:64, qs], bc_sb, op=MULT)
            if h2 == 1:
                s0 = sc * CW
                nc.sync.dma_start(out=ctx_m[m][64:128, s0:s0 + CW], in_=cno)

        def drain_chunk(sc, h, ps_c):
            drain_stage1(sc, h, ps_c, drain_stage0(sc, h, ps_c))

        # ---- prologue ----
        load_x("v", 0)
        load_x("k", 0)
        load_x("q", 0)
        load_x("v", 1)
        load_x("k", 1)
        load_x("q", 1)

        pro_slots = [(psX, "aux"), (psS, "s"), (psC, "c"), (psS, "s")]
        pro = [lambda p, t, st=st: v_round(st, p, t) for st in range(5)]
        pro += [lambda p, t: qk_round("k", 0, 0, p, t),
                lambda p, t: qk_round("q", 0, 0, p, t),
                lambda p, t: qk_round("q", 0, 1, p, t)]
        for i, fn in enumerate(pro):
            ps_pool, tg = pro_slots[i % len(pro_slots)]
            fn(ps_pool, tg)

        # ---- deferred per-chunk work (consumed one per sk) ----
        NCHUNK = NCH * HPC      # 12
        rounds = [[] for _ in range(NCHUNK)]

        def defer(ci, fn):
            rounds[ci].append(fn)

        aux = (psX, "aux")
        defer(0, lambda: (load_x("k", 2), load_x("v", 2)))
        defer(0, lambda: qk_round("k", 0, 1, *aux))
        defer(0, lambda: (load_x("k", 3), load_x("v", 3)))
        defer(0, lambda: v_round(5, *aux))
        defer(0, lambda: v_round(6, *aux))
        defer(0, lambda: v_round(7, *aux))
        defer(0, lambda: qk_round("k", 0, 2, *aux))
        defer(0, lambda: v_round(8, *aux))
        defer(0, lambda: v_round(9, *aux))
        defer(0, lambda: v_round(10, *aux))
        defer(0, lambda: qk_round("k", 0, 3, *aux))
        defer(0, lambda: v_round(11, *aux))
        defer(0, lambda: v_round(12, *aux))
        defer(0, lambda: v_round(13, *aux))
        defer(0, lambda: v_round(14, *aux))
        defer(0, lambda: v_round(15, *aux))
        defer(1, lambda: qk_round("k", 1, 0, *aux))
        defer(1, lambda: qk_round("k", 1, 1, *aux))
        defer(1, lambda: qk_round("q", 1, 0, *aux))
        defer(1, lambda: qk_round("q", 1, 1, *aux))
        defer(2, lambda: qk_round("k", 1, 2, *aux))
        defer(2, lambda: qk_round("k", 1, 3, *aux))
        defer(3, lambda: qk_round("k", 2, 0, *aux))
        defer(3, lambda: qk_round("k", 2, 1, *aux))
        defer(3, lambda: qk_round("q", 2, 0, *aux))
        defer(3, lambda: qk_round("q", 2, 1, *aux))
        defer(4, lambda: qk_round("k", 2, 2, *aux))
        defer(4, lambda: qk_round("k", 2, 3, *aux))
        defer(4, lambda: load_x("q", 2))
        defer(5, lambda: (load_x("q", 3), qk_round("q", 0, 2, *aux)))
        defer(5, lambda: qk_round("q", 0, 3, *aux))
        defer(6, lambda: qk_round("q", 1, 2, *aux))
        defer(6, lambda: qk_round("q", 1, 3, *aux))
        defer(7, lambda: qk_round("q", 2, 2, *aux))
        defer(7, lambda: qk_round("q", 2, 3, *aux))

        # ---- main attention loop ----
        pending_out = None
        pending_drain = None
        for ci in range(NCHUNK):
            sc, h = ci // HPC, ci % HPC
            m, h2 = h // 2, h % 2
            work = list(rounds[ci])
            wi = 0
            ps_c = None

            prev_e = None
            for sk in range(ST):
                sks = slice(sk * 128, (sk + 1) * 128)
                ps_s = psS.tile([128, CW], F32, tag="s", bufs=2,
                                name=f"ps{sc}{h}{sk}")
                for h4 in range(2):
                    sq = slice(sc * CW + h4 * 512, sc * CW + (h4 + 1) * 512)
                    qs = slice(h4 * 512, (h4 + 1) * 512)
                    nc.tensor.matmul(ps_s[:, qs],
                                     kt[m][h2 * 64:h2 * 64 + 64, sks],
                                     qt[m][h2 * 64:h2 * 64 + 64, sq])
                e = e_pool.tile([128, CW], BF16, tag="e", bufs=3,
                                name=f"e{sc}{h}{sk}")
                nc.scalar.activation(e, ps_s, EXP, scale=0.125)

                if prev_e is not None:
                    psk, pe = prev_e
                    if ps_c is None:
                        ps_c = psC.tile([128, CW], F32, tag="c",
                                        name=f"ctx{sc}{h}")
                    for half in range(2):
                        qs = slice(half * 512, (half + 1) * 512)
                        nc.tensor.matmul(ps_c[0:HEAD + 1, qs],
                                         vt[psk][:, h, :], pe[:, qs],
                                         start=(psk == 0), stop=False,
                                         skip_group_check=True)
                prev_e = (sk, e)

                if pending_drain is not None and sk <= 1:
                    if sk == 0:
                        _psc, _ph, _psc_r = pending_drain
                        pending_drain = (_psc, _ph,
                                         drain_stage0(_psc[0], _ph, _psc[1]))
                    else:
                        (_sc_ps, _ph, _r) = pending_drain
                        drain_stage1(_sc_ps[0], _ph, _sc_ps[1], _r)
                        pending_drain = None
                elif wi < len(work):
                    work[wi]()
                    wi += 1
                elif pending_out is not None:
                    if next(pending_out, _DONE) is _DONE:
                        pending_out = None

            psk, pe = prev_e
            for half in range(2):
                qs = slice(half * 512, (half + 1) * 512)
                nc.tensor.matmul(ps_c[0:HEAD + 1, qs],
                                 vt[psk][:, h, :], pe[:, qs],
                                 start=False, stop=True,
                                 skip_group_check=True)
            while wi < len(work):
                work[wi]()
                wi += 1
            if ci < NCHUNK - 1:
                pending_drain = ((sc, ps_c), h, None)
            else:
                drain_chunk(sc, h, ps_c)
            if ci == HPC:
                # sc=0 ctx complete once chunk 5's drain runs (chunk 6 sk0)
                pending_out = outproj_rounds(0)

        # tail: outproj(1); scores + bcast banks free -> deep rotation
        if pending_out is not None:
            for _ in pending_out:
                pass
        tail_slots = [(psX, "aux"), (psS, "s"), (psS, "s"), (psB, "b")]
        for _ in outproj_rounds(NCH - 1, slots=tail_slots):
            pass


def kernel(query, key, value, wq, bq, wk, bk, wv, bv, wo, bo):
    global _NC, LAST_RESULTS, _LAST_IN_MAPS
    if _NC is None:
        _NC = _build()

    import ml_dtypes

    def f32c(a):
        return np.ascontiguousarray(np.asarray(a, dtype=np.float32))

    def bf16c(a):
        return np.ascontiguousarray(
            np.asarray(a, dtype=np.float32).astype(ml_dtypes.bfloat16))

    query, key, value = map(np.asarray, (query, key, value))
    xt = [{"xqt": bf16c(query[b].T), "xkt": bf16c(key[b].T),
           "xvt": bf16c(value[b].T)} for b in range(B)]
    wslices = []
    for g in range(2):
        cols = slice(g * MC, (g + 1) * MC)
        wslices.append({
            "wq": bf16c(np.asarray(wq)[:, cols]),
            "wk": bf16c(np.asarray(wk)[:, cols]),
            "wv": bf16c(np.asarray(wv)[:, cols]),
            "wo": bf16c(np.asarray(wo)[cols, :]),
            "bq": f32c(np.asarray(bq)[cols]),
            "bk": f32c(np.asarray(bk)[cols]),
        })
    in_maps = [dict(xt[c // 2], **wslices[c % 2]) for c in range(NCORES)]

    _LAST_IN_MAPS = in_maps
    res = run_bass_kernel_spmd(_NC, in_maps, core_ids=list(range(NCORES)))
    LAST_RESULTS = res

    corr = (np.asarray(bv, np.float64) @ np.asarray(wo, np.float64)
            + np.asarray(bo, np.float64)).astype(np.float32)
    y = np.empty((B, S, D), np.float32)
    for b in range(B):
        y[b] = res.results[2 * b]["out"] + res.results[2 * b + 1]["out"] + corr
    return y
